# revision 12
# baseline (speedup 1.0000x reference)
"""Trainium2 Bass kernel for nn_BaseRuleLearner.

Math (per batch element b, reference semantics):
  UM[b,i,v,l]      = sum_e U[b,l,e]  * ru[i,v,e]
  BM[b,i,n,m,j,k]  = sum_e Bf[b,j,k,e] * rb[i,n,m,e]
  scores[b,i,p]    = sum_v UM[b,i,v,perm[p,v]]
                   + sum_{n,m} BM[b,i,n,m,perm[p,n],perm[p,m]]
  merged[b,i]      = min_p scores[b,i,p]
  out[b,:]         = softmax_i(merged) @ one_hot([0,0,1,1])

Kernel strategy (pure data parallel over B across 8 cores, 512 b/core).
DMA instruction count is the scarce resource (~625ns serialized HWDGE
overhead per dma_start), so everything is packed into few, large,
rectangular transfers:

Stage 1 (PE, float32r): one matmul per jk-PAIR jp (jk = 2*jp+s); a
block-diagonal weight [128=(s,e), 72=(i, t=s*9+nm)] packs the two k=64
contractions of a pair into one k=128 matmul:
psum[72=(i,t), 512 b] = BM[b,i,nm, jk=2jp+s].  Same for unary
(l = 2*lp+s, rows (i, tu=s*3+v)).

Evac (DVE/ACT alternating): psum -> SG staging [72, (jp, b)] in SBUF.

Assembly (1 DMA per (chunk, i), 24 total): Q-row order r = t*32 + jp
(unary r = 576 + tu*4 + lp) makes SG[i*18+4c : +nt, :] and
qt[c][0:nt*32, i*512:+512] the same element stream: src iterates
(t, jp, b), dst iterates (row=t*32+jp, b).

Stage 2 (PE, float32r): scores^T per (i, b-tile): psum[128 b, 336 p]
accumulated over 5 k-chunks; lhsT = qt[c] slice (stationary), rhs = G
chunk [k, 336], the 0/1 permutation-gather matrix (host-built,
input-independent).

Final: DVE min over p (free axis), softmax over i=4 (free axis),
pair-sum into [128, 4] result tiles, one gathered DMA out.
"""

import itertools
import numpy as np

B, O, E = 4096, 8, 64
I, V = 4, 3
P = 336
N_CORES = 8
BC = B // N_CORES            # 512 batch per core
NJP = (O * O) // 2           # 32 jk-pairs
NLP = O // 2                 # 4 l-pairs
R_BIN = O * O * 9            # 576 binary rows of Q/G
R_TOT = R_BIN + O * V        # 600 total rows
K_CHUNKS = [(0, 128), (128, 256), (256, 384), (384, 512), (512, R_TOT)]
NBT = BC // 128              # b-tiles per core (4)
JBS = BC + 16                # padded jp-block stride in sg (separate DMA runs)
JPG = 4                      # jp's per input DMA group
NXG = NJP // JPG             # binary input groups (4)

_PERM = np.array(list(itertools.permutations(range(O), V)), dtype=np.int32)

_CACHED = {}


def _build_g_packed():
    """G[r, p] with binary rows r = (s*9+nm)*32 + jp (jk=2jp+s=j*8+k),
    1 iff perm[p,n]==j and perm[p,m]==k; unary rows
    r = 576 + (s*3+v)*4 + lp (l=2lp+s), 1 iff perm[p,v]==l.
    Packed for a single DMA into [128, 5*336]: col-block c holds G rows
    [128c : 128c+kc]."""
    g = np.zeros((R_TOT, P), np.float32)
    ar = np.arange(P)
    for n in range(V):
        for m in range(V):
            jk = _PERM[:, n] * O + _PERM[:, m]
            r = (jk % 2 * 9 + n * V + m) * NJP + jk // 2
            g[r, ar] = 1.0
    for v in range(V):
        l = _PERM[:, v]
        r = R_BIN + (l % 2 * V + v) * NLP + l // 2
        g[r, ar] = 1.0
    packed = np.zeros((128, len(K_CHUNKS) * P), np.float32)
    for c, (r0, r1) in enumerate(K_CHUNKS):
        packed[0 : r1 - r0, c * P : (c + 1) * P] = g[r0:r1]
    return packed


def _build_module():
    import concourse.tile as tile
    from concourse import bacc, mybir

    FP = mybir.dt.float32
    FR = mybir.dt.float32r
    BF = mybir.dt.bfloat16
    X = mybir.AxisListType.X
    nc = bacc.Bacc("TRN2", target_bir_lowering=False, debug=False)

    ab = nc.dram_tensor("ab", [128, NJP * BC], BF, kind="ExternalInput")
    au = nc.dram_tensor("au", [128, NLP * BC], BF, kind="ExternalInput")
    w = nc.dram_tensor("w", [128, 96], BF, kind="ExternalInput")
    gm = nc.dram_tensor("gm", [128, len(K_CHUNKS) * P], BF, kind="ExternalInput")
    out = nc.dram_tensor("out", [BC, 4], FP, kind="ExternalOutput")

    with tile.TileContext(nc) as tc:
        with (
            tc.tile_pool(name="wpool", bufs=1) as wpool,
            tc.tile_pool(name="xpool", bufs=3) as xpool,
            tc.tile_pool(name="sgpool", bufs=1) as sgpool,
            tc.tile_pool(name="qpool", bufs=1) as qpool,
            tc.tile_pool(name="mpool", bufs=2) as mpool,
            tc.tile_pool(name="psb", bufs=4, space="PSUM") as psb,
            tc.tile_pool(name="psu", bufs=1, space="PSUM") as psu,
            tc.tile_pool(name="pss", bufs=3, space="PSUM") as pss,
        ):
            # ---- phase 0: weights + G (one DMA each) ----
            w_sb = wpool.tile([128, 96], BF, tag="w")
            nc.sync.dma_start(w_sb[:], w.ap()[:])
            rb_sb = w_sb[:, 0:72]
            ru_sb = w_sb[:, 72:96]
            g_sb = wpool.tile([128, len(K_CHUNKS) * P], BF, tag="g")
            nc.sync.dma_start(g_sb[:], gm.ap()[:])

            qt = [
                [
                    qpool.tile(
                        [128, BC], BF, tag=f"q{c}_{i}", name=f"q{c}_{i}"
                    )
                    for i in range(I)
                ]
                for c in range(5)
            ]
            sg = sgpool.tile([72, NJP * JBS], BF, tag="sg")
            sgu = sgpool.tile([24, NLP * JBS], BF, tag="sgu")

            # ---- phase 1u: unary ----
            xu = xpool.tile([128, NLP * BC], BF, tag="xu")
            nc.sync.dma_start(xu[:], au.ap()[:])
            for lp in range(NLP):
                pu = psu.tile([24, BC], FP, tag="pu")
                nc.tensor.matmul(
                    pu[:],
                    ru_sb,
                    xu[:, lp * BC : (lp + 1) * BC],
                    start=True,
                    stop=True,
                )
                nc.vector.tensor_copy(sgu[:, lp * JBS : lp * JBS + BC], pu[:])

            # ---- phase 1: binary stage-1 matmuls + evac ----
            for xg in range(NXG):
                xt = xpool.tile([128, JPG * BC], BF, tag="x")
                nc.sync.dma_start(
                    xt[:], ab.ap()[:, xg * JPG * BC : (xg + 1) * JPG * BC]
                )
                for jl in range(JPG):
                    jp = xg * JPG + jl
                    pb = psb.tile([72, BC], FP, tag="pb")
                    nc.tensor.matmul(
                        pb[:],
                        rb_sb,
                        xt[:, jl * BC : (jl + 1) * BC],
                        start=True,
                        stop=True,
                    )
                    dst = sg[:, jp * JBS : jp * JBS + BC]
                    if jp % 2 == 0:
                        nc.vector.tensor_copy(dst, pb[:])
                    else:
                        nc.scalar.copy(dst, pb[:])

            # ---- assembly: 1 DMA per (chunk, i); padded src runs ----
            for i in range(I):
                srcvu = (
                    sgu[i * 6 : i * 6 + 6, :]
                    .rearrange("p (a m) -> p a m", m=JBS)[:, :, 0:BC]
                )
                nc.gpsimd.dma_start(qt[4][i][64 : 64 + 24, :], srcvu)
            for i in range(I):
                for c in range(5):
                    nt = 4 if c < 4 else 2
                    srcv = (
                        sg[i * 18 + 4 * c : i * 18 + 4 * c + nt, :]
                        .rearrange("p (a m) -> p a m", m=JBS)[:, :, 0:BC]
                    )
                    eng = nc.gpsimd if (i * 5 + c) % 2 == 0 else nc.sync
                    eng.dma_start(qt[c][i][0 : nt * 32, :], srcv)

            # ---- phase 2: scores + min + softmax ----
            fin = mpool.tile([128, 4 * NBT], FP, tag="fin", bufs=1)
            for bt in range(NBT):
                merged = mpool.tile([128, 4], FP, tag="m")
                for i in range(I):
                    sc = pss.tile([128, P], FP, tag="sc")
                    col = bt * 128
                    for c, (r0, r1) in enumerate(K_CHUNKS):
                        kc = r1 - r0
                        nc.tensor.matmul(
                            sc[:],
                            qt[c][i][0:kc, col : col + 128],
                            g_sb[0:kc, c * P : (c + 1) * P],
                            start=(c == 0),
                            stop=(c == len(K_CHUNKS) - 1),
                        )
                    nc.vector.tensor_reduce(
                        merged[:, i : i + 1], sc[:], axis=X, op=mybir.AluOpType.min
                    )
                mx = mpool.tile([128, 1], FP, tag="mx")
                nc.vector.tensor_reduce(
                    mx[:], merged[:], axis=X, op=mybir.AluOpType.max
                )
                sh = mpool.tile([128, 4], FP, tag="sh")
                nc.vector.tensor_scalar_sub(sh[:], merged[:], mx[:])
                ex = mpool.tile([128, 4], FP, tag="ex")
                sm = mpool.tile([128, 1], FP, tag="sm")
                nc.scalar.activation(
                    ex[:], sh[:], mybir.ActivationFunctionType.Exp, accum_out=sm[:]
                )
                rc = mpool.tile([128, 1], FP, tag="rc")
                nc.vector.reciprocal(rc[:], sm[:])
                pr = mpool.tile([128, 4], FP, tag="pr")
                nc.vector.tensor_scalar_mul(pr[:], ex[:], rc[:])
                pr3 = pr[:].rearrange("p (a b) -> p a b", b=2)
                nc.vector.tensor_add(
                    fin[:, bt * 4 : bt * 4 + 2], pr3[:, :, 0], pr3[:, :, 1]
                )
                nc.vector.memset(fin[:, bt * 4 + 2 : bt * 4 + 4], 0.0)
            # single gathered output DMA: out[bt*128 + q, col] = fin[q, bt*4+col]
            outv = out.ap().rearrange("(a p) m -> p a m", p=128)  # [128, NBT, 4]
            nc.sync.dma_start(outv, fin[:].rearrange("p (a m) -> p a m", a=NBT))

    nc.compile()
    return nc


def _get_module():
    if "nc" not in _CACHED:
        _CACHED["nc"] = _build_module()
    return _CACHED["nc"]


def _host_inputs(unary_feats, binary_feats, rule_unary, rule_binary):
    """Shard + lay out inputs for the 8 cores."""
    import ml_dtypes

    bf16 = ml_dtypes.bfloat16
    uf = np.asarray(unary_feats, dtype=np.float32).astype(bf16)
    bf = np.asarray(binary_feats, dtype=np.float32).astype(bf16)
    ru = np.asarray(rule_unary, dtype=np.float32).astype(bf16)
    rb = np.asarray(rule_binary, dtype=np.float32).astype(bf16)

    rb_flat = rb.transpose(3, 0, 1, 2).reshape(E, I * 9)   # [e, (i,nm)]
    ru_flat = ru.transpose(2, 0, 1).reshape(E, I * V)      # [e, (i,v)]
    w = np.zeros((128, 96), bf16)
    for s in range(2):
        for i in range(I):
            w[s * 64 : (s + 1) * 64, i * 18 + s * 9 : i * 18 + s * 9 + 9] = (
                rb_flat[:, i * 9 : (i + 1) * 9]
            )
            w[s * 64 : (s + 1) * 64, 72 + i * 6 + s * 3 : 72 + i * 6 + s * 3 + 3] = (
                ru_flat[:, i * 3 : (i + 1) * 3]
            )
    g = _build_g_packed().astype(bf16)

    in_maps = []
    for c in range(N_CORES):
        bfc = bf[c * BC : (c + 1) * BC]                    # [BC, O, O, E]
        x = bfc.reshape(BC, O * O, E).transpose(1, 2, 0)   # [jk, e, b]
        ab = np.ascontiguousarray(
            x.reshape(NJP, 2, E, BC).transpose(1, 2, 0, 3)
        ).reshape(128, NJP * BC)                           # [(s,e), (jp,b)]
        ufc = uf[c * BC : (c + 1) * BC]                    # [BC, O, E]
        xu = ufc.transpose(1, 2, 0)                        # [l, e, b]
        au = np.ascontiguousarray(
            xu.reshape(NLP, 2, E, BC).transpose(1, 2, 0, 3)
        ).reshape(128, NLP * BC)                           # [(s,e), (lp,b)]
        in_maps.append({"ab": ab, "au": au, "w": w, "gm": g})
    return in_maps


TRACE = False  # set True (e.g. from test.py) to capture an NTFF profile


def kernel(unary_feats, binary_feats, rule_unary, rule_binary):
    from concourse.bass_utils import run_bass_kernel_spmd

    nc = _get_module()
    in_maps = _host_inputs(unary_feats, binary_feats, rule_unary, rule_binary)
    res = run_bass_kernel_spmd(
        nc, in_maps, core_ids=list(range(N_CORES)), trace=TRACE
    )
    _CACHED["last_results"] = res
    return np.concatenate(
        [res.results[c]["out"] for c in range(N_CORES)], axis=0
    )


# revision 15
# speedup vs baseline: 1.0711x; 1.0711x over previous
"""Trainium2 Bass kernel for nn_BaseRuleLearner.

Math (per batch element b, reference semantics):
  UM[b,i,v,l]      = sum_e U[b,l,e]  * ru[i,v,e]
  BM[b,i,n,m,j,k]  = sum_e Bf[b,j,k,e] * rb[i,n,m,e]
  scores[b,i,p]    = sum_v UM[b,i,v,perm[p,v]]
                   + sum_{n,m} BM[b,i,n,m,perm[p,n],perm[p,m]]
  merged[b,i]      = min_p scores[b,i,p]
  out[b,:]         = softmax_i(merged) @ one_hot([0,0,1,1])

Kernel strategy (pure data parallel over B across 8 cores, 512 b/core).
DMA instruction count is the scarce resource (~625ns serialized HWDGE
overhead per dma_start), so everything is packed into few, large,
rectangular transfers:

Stage 1 (PE, float32r): one matmul per jk-PAIR jp (jk = 2*jp+s); a
block-diagonal weight [128=(s,e), 72=(i, t=s*9+nm)] packs the two k=64
contractions of a pair into one k=128 matmul:
psum[72=(i,t), 512 b] = BM[b,i,nm, jk=2jp+s].  Same for unary
(l = 2*lp+s, rows (i, tu=s*3+v)).

Evac (DVE/ACT alternating): psum -> SG staging [72, (jp, b)] in SBUF.

Assembly (1 DMA per (chunk, i), 24 total): Q-row order r = t*32 + jp
(unary r = 576 + tu*4 + lp) makes SG[i*18+4c : +nt, :] and
qt[c][0:nt*32, i*512:+512] the same element stream: src iterates
(t, jp, b), dst iterates (row=t*32+jp, b).

Stage 2 (PE, float32r): scores^T per (i, b-tile): psum[128 b, 336 p]
accumulated over 5 k-chunks; lhsT = qt[c] slice (stationary), rhs = G
chunk [k, 336], the 0/1 permutation-gather matrix (host-built,
input-independent).

Final: DVE min over p (free axis), softmax over i=4 (free axis),
pair-sum into [128, 4] result tiles, one gathered DMA out.
"""

import itertools
import numpy as np

B, O, E = 4096, 8, 64
I, V = 4, 3
P = 336
N_CORES = 8
BC = B // N_CORES            # 512 batch per core
NJP = (O * O) // 2           # 32 jk-pairs
NLP = O // 2                 # 4 l-pairs
R_ND = 12 * 32               # off-diag rows of Q/G (t'' major, jp minor)
R_DG = 24                    # diag rows: (s,nmd) x 4 used jp
R_UN = 24                    # unary rows
R_TOT = R_ND + R_DG + R_UN   # 456 total rows
K_CHUNKS = [(0, 128), (128, 256), (256, 384), (384, R_TOT)]
OD_IX = {1: 0, 2: 1, 3: 2, 5: 3, 6: 4, 7: 5}   # offdiag nm -> 0..5
DG_IX = {0: 0, 4: 1, 8: 2}                     # diag nm -> 0..2
NBT = BC // 128              # b-tiles per core (4)
JBS = BC + 16                # padded jp-block stride in sg (separate DMA runs)
JPG = 4                      # jp's per input DMA group
NXG = NJP // JPG             # binary input groups (4)

_PERM = np.array(list(itertools.permutations(range(O), V)), dtype=np.int32)

_CACHED = {}


def _build_g_packed():
    """G[r, p] in the pruned, t''-major layout:
    off-diag rows r = (s*6 + OD_IX[nm])*32 + jp for jk=2jp+s=j*8+k;
    diag rows (j==k only) r = 384 + (s*3 + DG_IX[nm])*4 + jpi;
    unary rows r = 408 + (s*3 + v)*4 + lp (l=2lp+s).
    Packed into [128, 4*336]: col-block c holds G rows [128c : 128c+kc]."""
    g = np.zeros((R_TOT, P), np.float32)
    ar = np.arange(P)
    for n in range(V):
        for m in range(V):
            nm = n * V + m
            jk = _PERM[:, n] * O + _PERM[:, m]
            s, jp = jk % 2, jk // 2
            if nm in OD_IX:
                r = (s * 6 + OD_IX[nm]) * NJP + jp
                g[r, ar] = 1.0
            else:
                mask = _PERM[:, n] == _PERM[:, m]
                jpi = (jp[mask] - 4 * s[mask]) // 9
                r = R_ND + (s[mask] * 3 + DG_IX[nm]) * 4 + jpi
                g[r, ar[mask]] = 1.0
    for v in range(V):
        l = _PERM[:, v]
        r = R_ND + R_DG + (l % 2 * V + v) * NLP + l // 2
        g[r, ar] = 1.0
    packed = np.zeros((128, len(K_CHUNKS) * P), np.float32)
    for c, (r0, r1) in enumerate(K_CHUNKS):
        packed[0 : r1 - r0, c * P : (c + 1) * P] = g[r0:r1]
    return packed


def _build_module():
    import concourse.tile as tile
    from concourse import bacc, mybir

    FP = mybir.dt.float32
    FR = mybir.dt.float32r
    BF = mybir.dt.bfloat16
    X = mybir.AxisListType.X
    nc = bacc.Bacc("TRN2", target_bir_lowering=False, debug=False)

    ab = nc.dram_tensor("ab", [128, NJP * BC], BF, kind="ExternalInput")
    au = nc.dram_tensor("au", [128, NLP * BC], BF, kind="ExternalInput")
    w = nc.dram_tensor("w", [128, 96], BF, kind="ExternalInput")
    gm = nc.dram_tensor("gm", [128, len(K_CHUNKS) * P], BF, kind="ExternalInput")
    out = nc.dram_tensor("out", [BC, 4], FP, kind="ExternalOutput")

    with tile.TileContext(nc) as tc:
        with (
            tc.tile_pool(name="wpool", bufs=1) as wpool,
            tc.tile_pool(name="xpool", bufs=3) as xpool,
            tc.tile_pool(name="sgpool", bufs=1) as sgpool,
            tc.tile_pool(name="qpool", bufs=1) as qpool,
            tc.tile_pool(name="mpool", bufs=2) as mpool,
            tc.tile_pool(name="psb", bufs=4, space="PSUM") as psb,
            tc.tile_pool(name="psu", bufs=1, space="PSUM") as psu,
            tc.tile_pool(name="pss", bufs=3, space="PSUM") as pss,
        ):
            # ---- phase 0: weights + G (one DMA each) ----
            w_sb = wpool.tile([128, 96], BF, tag="w")
            nc.sync.dma_start(w_sb[:], w.ap()[:])
            rb_sb = w_sb[:, 0:72]
            ru_sb = w_sb[:, 72:96]
            g_sb = wpool.tile([128, len(K_CHUNKS) * P], BF, tag="g")
            nc.sync.dma_start(g_sb[:], gm.ap()[:])

            qt = [
                [
                    qpool.tile(
                        [128, BC], BF, tag=f"q{c}_{i}", name=f"q{c}_{i}"
                    )
                    for i in range(I)
                ]
                for c in range(4)
            ]
            sg = sgpool.tile([72, NJP * JBS], BF, tag="sg")
            sgu = sgpool.tile([24, NLP * JBS], BF, tag="sgu")

            # ---- phase 1u: unary ----
            xu = xpool.tile([128, NLP * BC], BF, tag="xu")
            nc.sync.dma_start(xu[:], au.ap()[:])
            for lp in range(NLP):
                pu = psu.tile([24, BC], FP, tag="pu")
                nc.tensor.matmul(
                    pu[:],
                    ru_sb,
                    xu[:, lp * BC : (lp + 1) * BC],
                    start=True,
                    stop=True,
                )
                nc.vector.tensor_copy(sgu[:, lp * JBS : lp * JBS + BC], pu[:])

            # ---- phase 1: binary stage-1 matmuls + evac ----
            for xg in range(NXG):
                xt = xpool.tile([128, JPG * BC], BF, tag="x")
                nc.sync.dma_start(
                    xt[:], ab.ap()[:, xg * JPG * BC : (xg + 1) * JPG * BC]
                )
                for jl in range(JPG):
                    jp = xg * JPG + jl
                    pb = psb.tile([72, BC], FP, tag="pb")
                    nc.tensor.matmul(
                        pb[:],
                        rb_sb,
                        xt[:, jl * BC : (jl + 1) * BC],
                        start=True,
                        stop=True,
                    )
                    dst = sg[:, jp * JBS : jp * JBS + BC]
                    if jp % 2 == 0:
                        nc.vector.tensor_copy(dst, pb[:])
                    else:
                        nc.scalar.copy(dst, pb[:])

            # ---- assembly: 1 DMA per (chunk, i); padded src runs ----
            for i in range(I):
                srcvu = (
                    sgu[i * 6 : i * 6 + 6, :]
                    .rearrange("p (a m) -> p a m", m=JBS)[:, :, 0:BC]
                )
                nc.gpsimd.dma_start(qt[3][i][R_DG : R_DG + R_UN, :], srcvu)
            for i in range(I):
                for s in range(2):
                    # diag rows: src t'' = 12 + s*3 .. +3, jp in {4s, 4s+9, ...}
                    srcd = (
                        sg[i * 18 + 12 + s * 3 : i * 18 + 12 + s * 3 + 3, :]
                        .rearrange("p (a m) -> p a m", m=JBS)
                        [:, 4 * s : 4 * s + 28 : 9, 0:BC]
                    )
                    nc.gpsimd.dma_start(
                        qt[3][i][s * 12 : s * 12 + 12, :], srcd
                    )
                for c in range(3):
                    srcv = (
                        sg[i * 18 + 4 * c : i * 18 + 4 * c + 4, :]
                        .rearrange("p (a m) -> p a m", m=JBS)[:, :, 0:BC]
                    )
                    nc.gpsimd.dma_start(qt[c][i][:, :], srcv)

            # ---- phase 2: scores + min + softmax ----
            fin = mpool.tile([128, 4 * NBT], FP, tag="fin", bufs=1)
            for bt in range(NBT):
                merged = mpool.tile([128, 4], FP, tag="m")
                for i in range(I):
                    sc = pss.tile([128, P], FP, tag="sc")
                    col = bt * 128
                    for c, (r0, r1) in enumerate(K_CHUNKS):
                        kc = r1 - r0
                        nc.tensor.matmul(
                            sc[:],
                            qt[c][i][0:kc, col : col + 128],
                            g_sb[0:kc, c * P : (c + 1) * P],
                            start=(c == 0),
                            stop=(c == len(K_CHUNKS) - 1),
                        )
                    nc.vector.tensor_reduce(
                        merged[:, i : i + 1], sc[:], axis=X, op=mybir.AluOpType.min
                    )
                mx = mpool.tile([128, 1], FP, tag="mx")
                nc.vector.tensor_reduce(
                    mx[:], merged[:], axis=X, op=mybir.AluOpType.max
                )
                sh = mpool.tile([128, 4], FP, tag="sh")
                nc.vector.tensor_scalar_sub(sh[:], merged[:], mx[:])
                ex = mpool.tile([128, 4], FP, tag="ex")
                sm = mpool.tile([128, 1], FP, tag="sm")
                nc.scalar.activation(
                    ex[:], sh[:], mybir.ActivationFunctionType.Exp, accum_out=sm[:]
                )
                rc = mpool.tile([128, 1], FP, tag="rc")
                nc.vector.reciprocal(rc[:], sm[:])
                pr = mpool.tile([128, 4], FP, tag="pr")
                nc.vector.tensor_scalar_mul(pr[:], ex[:], rc[:])
                pr3 = pr[:].rearrange("p (a b) -> p a b", b=2)
                nc.vector.tensor_add(
                    fin[:, bt * 4 : bt * 4 + 2], pr3[:, :, 0], pr3[:, :, 1]
                )
                nc.vector.memset(fin[:, bt * 4 + 2 : bt * 4 + 4], 0.0)
            # single gathered output DMA: out[bt*128 + q, col] = fin[q, bt*4+col]
            outv = out.ap().rearrange("(a p) m -> p a m", p=128)  # [128, NBT, 4]
            nc.sync.dma_start(outv, fin[:].rearrange("p (a m) -> p a m", a=NBT))

    nc.compile()
    return nc


def _get_module():
    if "nc" not in _CACHED:
        _CACHED["nc"] = _build_module()
    return _CACHED["nc"]


def _host_inputs(unary_feats, binary_feats, rule_unary, rule_binary):
    """Shard + lay out inputs for the 8 cores."""
    import ml_dtypes

    bf16 = ml_dtypes.bfloat16
    uf = np.asarray(unary_feats, dtype=np.float32).astype(bf16)
    bf = np.asarray(binary_feats, dtype=np.float32).astype(bf16)
    ru = np.asarray(rule_unary, dtype=np.float32).astype(bf16)
    rb = np.asarray(rule_binary, dtype=np.float32).astype(bf16)

    rb_flat = rb.transpose(3, 0, 1, 2).reshape(E, I * 9)   # [e, (i,nm)]
    ru_flat = ru.transpose(2, 0, 1).reshape(E, I * V)      # [e, (i,v)]
    w = np.zeros((128, 96), bf16)
    for s in range(2):
        for i in range(I):
            for nm in range(9):
                t2 = s * 6 + OD_IX[nm] if nm in OD_IX else 12 + s * 3 + DG_IX[nm]
                w[s * 64 : (s + 1) * 64, i * 18 + t2] = rb_flat[:, i * 9 + nm]
            w[s * 64 : (s + 1) * 64, 72 + i * 6 + s * 3 : 72 + i * 6 + s * 3 + 3] = (
                ru_flat[:, i * 3 : (i + 1) * 3]
            )
    g = _build_g_packed().astype(bf16)

    in_maps = []
    for c in range(N_CORES):
        bfc = bf[c * BC : (c + 1) * BC]                    # [BC, O, O, E]
        x = bfc.reshape(BC, O * O, E).transpose(1, 2, 0)   # [jk, e, b]
        ab = np.ascontiguousarray(
            x.reshape(NJP, 2, E, BC).transpose(1, 2, 0, 3)
        ).reshape(128, NJP * BC)                           # [(s,e), (jp,b)]
        ufc = uf[c * BC : (c + 1) * BC]                    # [BC, O, E]
        xu = ufc.transpose(1, 2, 0)                        # [l, e, b]
        au = np.ascontiguousarray(
            xu.reshape(NLP, 2, E, BC).transpose(1, 2, 0, 3)
        ).reshape(128, NLP * BC)                           # [(s,e), (lp,b)]
        in_maps.append({"ab": ab, "au": au, "w": w, "gm": g})
    return in_maps


TRACE = False  # set True (e.g. from test.py) to capture an NTFF profile


def kernel(unary_feats, binary_feats, rule_unary, rule_binary):
    from concourse.bass_utils import run_bass_kernel_spmd

    nc = _get_module()
    in_maps = _host_inputs(unary_feats, binary_feats, rule_unary, rule_binary)
    res = run_bass_kernel_spmd(
        nc, in_maps, core_ids=list(range(N_CORES)), trace=TRACE
    )
    _CACHED["last_results"] = res
    return np.concatenate(
        [res.results[c]["out"] for c in range(N_CORES)], axis=0
    )
